# revision 22
# baseline (speedup 1.0000x reference)
"""Trainium2 Bass kernel for nn_CombinedOrthogonalAdapter (MoE-routed LoRA).

Math (per token t): out[t, :] = (x[t, :] @ A_e^T) @ B_e^T,  e = task_indices[t]
with E=8 experts, rank R=64, D=2048, B*S = 16384 tokens, SCALE = 1.0.

Strategy (v3, expert-parallel, host-routed, bf16):
  - Routing is pure data movement, so it happens on host (numpy argsort),
    like the host-side transpose the v1 kernel already did. Core c gets ALL
    tokens of expert c (max count 2168 for this input), padded to NSLOT
    slots, pre-gathered AND pre-transposed: xgT [D, NSLOT] in bf16.
  - Device per core: two dense GEMMs with only its own expert's weights:
      stage A:  H^T[r, s]    = sum_d A_e[r, d] * xgT[d, s]   (PSUM acc over d)
      stage B:  yT[dout, s]  = sum_r B_e[dout, r] * H^T[r, s]
    All matmul inputs bf16 (1 cycle/row on PE), PSUM fp32, evictions cast
    back to bf16. Output yT [D, NSLOT] bf16; host scatters tokens back and
    casts fp32.
  - DMA is the bottleneck in the cost model (all transfers serialize at
    ~360 GB/s/core): bf16 halves traffic vs fp32 -> ~18 MB/core ~ 50 us.
    Tokens are processed in two column groups so stage-B output DMA of
    group 0 overlaps the stage-A input DMA of group 1.
"""

import os

import numpy as np
from ml_dtypes import bfloat16

import concourse.bacc as bacc
import concourse.mybir as mybir
import concourse.tile as tile
from concourse.bass_utils import run_bass_kernel_spmd

# Problem shapes (hardcoded per contest rules).
B, S, D, E, R = 4, 4096, 2048, 8, 64
N_TOK = B * S                     # 16384
N_CORES = 8
DCH = D // 128                    # 16 d chunks

F32 = mybir.dt.float32
BF16 = mybir.dt.bfloat16

LAST_RESULTS = None               # test.py introspection hook
_BUILD_CACHE = {}


def _col_tiles(nslot):
    """[(col0, width)] with width <= 512 (one PSUM bank of fp32)."""
    out = []
    c = 0
    while c < nslot:
        w = min(512, nslot - c)
        out.append((c, w))
        c += w
    return out


def _groups(colt):
    """Split col tiles into two pipeline groups, ~60/40: group 0 is larger
    so its output DMAs cover group 1's stage-A tail + eviction latency."""
    cut = int(sum(w for _, w in colt) * 0.72)
    acc, g0 = 0, []
    for i, (_, w) in enumerate(colt):
        if g0 and acc + w > cut:
            break
        g0.append(i)
        acc += w
    g1 = [i for i in range(len(colt)) if i not in g0]
    return [g0, g1] if g1 else [g0]


def _build(nslot):
    nc = bacc.Bacc(
        "TRN2",
        target_bir_lowering=False,
        debug=False,
        enable_asserts=False,
        num_devices=N_CORES,
    )

    colt = _col_tiles(nslot)
    groups = _groups(colt)

    xgt_d = nc.dram_tensor("xgt", [D, nslot], BF16, kind="ExternalInput")
    # aT packed: ap[p, cd*64 + r] = A_e[r, cd*128 + p]
    a_d = nc.dram_tensor("ap", [128, DCH * R], BF16, kind="ExternalInput")
    # bT: bt[r, dout] = B_e[dout, r]
    b_d = nc.dram_tensor("bt", [R, D], BF16, kind="ExternalInput")
    y_d = nc.dram_tensor("yg", [nslot, D], BF16, kind="ExternalOutput")

    with tile.TileContext(nc) as tc:
        with (
            tc.tile_pool(name="wpool", bufs=1) as wpool,
            tc.tile_pool(name="xpool", bufs=1) as xpool,
            tc.tile_pool(name="hpool", bufs=1) as hpool,
            tc.tile_pool(name="ypool", bufs=1) as ypool,
            tc.tile_pool(name="psA", bufs=1, space="PSUM") as psA,
            tc.tile_pool(name="psB", bufs=3, space="PSUM") as psB,
        ):
            a_sb = wpool.tile([128, DCH * R], BF16, name="a_sb", tag="a_sb")
            nc.sync.dma_start(a_sb[:], a_d[:, :])
            b_sb = wpool.tile([R, D], BF16, name="b_sb", tag="b_sb")
            nc.sync.dma_start(b_sb[:], b_d[:, :])

            # group geometry
            gcol = []            # (col0, width) per group
            for g in groups:
                c0 = colt[g[0]][0]
                w = sum(colt[j][1] for j in g)
                gcol.append((c0, w))

            # input DMAs for all groups up-front (program order = DMA order);
            # small group-1 transfers alternate issue queues (SP/ACT) so the
            # issue rate is not the limiter
            xg_sb = {}
            for gi, g in enumerate(groups):
                c0, gw = gcol[gi]
                for cd in range(DCH):
                    xt = xpool.tile([128, gw], BF16, name=f"x_{gi}_{cd}",
                                    tag=f"x_{gi}_{cd}")
                    nc.sync.dma_start(
                        xt[:], xgt_d[cd * 128:(cd + 1) * 128, c0:c0 + gw])
                    xg_sb[(gi, cd)] = xt

            for gi, g in enumerate(groups):
                c0, gw = gcol[gi]
                # ---- stage A: H^T[r, cols] accumulated over d chunks ----
                hps = {}
                for j in g:
                    jc0, jw = colt[j]
                    hps[j] = psA.tile([R, jw], F32, name=f"hps{j}",
                                      tag=f"hps{j}")
                for cd in range(DCH):
                    xt = xg_sb[(gi, cd)]
                    for j in g:
                        jc0, jw = colt[j]
                        l0 = jc0 - c0
                        nc.tensor.matmul(
                            hps[j][:],
                            lhsT=a_sb[:, cd * R:(cd + 1) * R],
                            rhs=xt[:, l0:l0 + jw],
                            start=(cd == 0),
                            stop=(cd == DCH - 1),
                        )
                h_sb = hpool.tile([R, gw], BF16, name=f"h_sb{gi}",
                                  tag=f"h_sb{gi}")
                for k, j in enumerate(g):
                    jc0, jw = colt[j]
                    l0 = jc0 - c0
                    if k % 2 == 0:
                        nc.vector.tensor_copy(h_sb[:, l0:l0 + jw], hps[j][:])
                    else:
                        nc.scalar.copy(h_sb[:, l0:l0 + jw], hps[j][:])

                # ---- stage B: yT[dout, cols] = B_e @ H ----
                # ---- stage B: y[slot, dout] per 128-token slot chunk ----
                for sc in range(gw // 128):
                    s0 = c0 + sc * 128          # global slot base
                    l0 = sc * 128               # group-local slot base
                    y_sb = ypool.tile([128, D], BF16, name="y_sb",
                                      tag="y_sb", bufs=5)
                    # evictions of one chunk rotate across engines so they
                    # run in parallel and keep the out-DMA fed
                    for k in range(D // 512):
                        yps = psB.tile([128, 512], F32, name="yps", tag="yps")
                        nc.tensor.matmul(
                            yps[:],
                            lhsT=h_sb[:, l0:l0 + 128],
                            rhs=b_sb[:, k * 512:(k + 1) * 512],
                            start=True,
                            stop=True,
                        )
                        # GPSIMD cannot read PSUM (BIR verifier) -> DVE/ACT
                        if k % 2 == 0:
                            nc.vector.tensor_copy(
                                y_sb[:, k * 512:(k + 1) * 512], yps[:])
                        else:
                            nc.scalar.copy(
                                y_sb[:, k * 512:(k + 1) * 512], yps[:])
                    nc.sync.dma_start(y_d[s0:s0 + 128, :], y_sb[:])
    nc.compile()
    return nc


def _route(task_indices):
    """Host-side routing: per-expert token index lists (stable order)."""
    idx = np.asarray(task_indices).reshape(-1).astype(np.int64)
    order = np.argsort(idx, kind="stable")
    sorted_idx = idx[order]
    starts = np.searchsorted(sorted_idx, np.arange(E + 1))
    perms = [order[starts[e]:starts[e + 1]] for e in range(E)]
    return perms


def prepare_in_maps(x, lora_A, lora_B, task_indices):
    xf = np.asarray(x, dtype=np.float32).reshape(N_TOK, D)
    lora_A = np.asarray(lora_A, dtype=np.float32)
    lora_B = np.asarray(lora_B, dtype=np.float32)
    perms = _route(task_indices)
    max_cnt = max(len(p) for p in perms)
    nslot = ((max_cnt + 127) // 128) * 128

    in_maps = []
    for e in range(E):
        p = perms[e]
        xg = np.zeros((nslot, D), dtype=bfloat16)
        xg[:len(p)] = xf[p]
        xgt = np.ascontiguousarray(xg.T)                    # [D, nslot]
        ap = np.ascontiguousarray(
            lora_A[e].T.reshape(DCH, 128, R).transpose(1, 0, 2)
            .reshape(128, DCH * R).astype(bfloat16))        # [128, DCH*R]
        bt = np.ascontiguousarray(lora_B[e].T.astype(bfloat16))  # [R, D]
        in_maps.append({"xgt": xgt, "ap": ap, "bt": bt})
    return in_maps, perms, nslot


_LAST_NSLOT = 2176


def _get_nc(nslot=None):
    if nslot is None:
        nslot = _LAST_NSLOT
    if nslot not in _BUILD_CACHE:
        _BUILD_CACHE[nslot] = _build(nslot)
    return _BUILD_CACHE[nslot]


def kernel(x, lora_A, lora_B, task_indices):
    global LAST_RESULTS, _LAST_NSLOT
    in_maps, perms, nslot = prepare_in_maps(x, lora_A, lora_B, task_indices)
    _LAST_NSLOT = nslot
    nc = _get_nc(nslot)
    res = run_bass_kernel_spmd(
        nc, in_maps, core_ids=list(range(N_CORES)),
        trace=bool(int(os.environ.get("KERNEL_TRACE", "0"))),
    )
    LAST_RESULTS = res

    out = np.empty((N_TOK, D), dtype=np.float32)
    for e in range(E):
        p = perms[e]
        yg = np.asarray(res.results[e]["yg"])               # [nslot, D] bf16
        out[p] = yg[:len(p)].astype(np.float32)
    return out.reshape(B, S, D)
